# revision 6
# baseline (speedup 1.0000x reference)
"""GNO message-passing kernel for Trainium2 (8 NeuronCores, edge-parallel).

Math (matches the reference):
    h  = relu(relu(relu(ea@W1+b1)@W2+b2)@W3+b3)
    w  = (h@W4+b4).reshape(E,16,16)
    msg= einsum('ei,eio->eo', x[src], w)
    agg= segment_mean(msg, dst, N)
    out= x@root + agg + bias

Strategy (v3):
  - Edges are split into 8 contiguous shards (one per core) and sorted by
    dst on the host.  The host pre-gathers x[src], and also computes the
    first MLP layer h1 = relu(ea@W1+b1) (cheap BLAS), so the device sees
    dense fp16 streams: h1 (tile-major) and gathered source features.
  - Per 512-edge tile on-device: W2/W3/W4 on TensorE (bias-as-extra-row
    trick for layer 4), relu2/relu3 on Scalar (Act), per-edge einsum:
    DVE broadcast-multiply from PSUM, then a pairwise fp16 add-tree for
    the 16->1 reduction -- first level on GpSimd (Pool), rest on DVE
    where the all-SBUF fp16 2x fast path applies.  Per-edge messages are
    written back densely in fp16 -- no on-device scatter at all.
  - x@root+bias is column-streamed through one stationary [17,16] load.
  - Host: np.add.reduceat over dst-sorted runs, divide by counts, add
    the root part.
"""

import math
import numpy as np

import concourse.bass as bass
import concourse.bacc as bacc
import concourse.mybir as mybir
import concourse.tile as tile
from concourse.bass_utils import run_bass_kernel_spmd

FP16 = np.float16

N_NODES = 50000
N_EDGES = 800000
N_CORES = 8
ETILE = 512
TB = 4                      # tiles per DMA batch
P = 128
ESH = N_EDGES // N_CORES    # 100000 edges per core
T = math.ceil(ESH / ETILE)  # 196 tiles
NB = T // TB                # 49 batches
EP = T * ETILE              # 100352 padded edges
NSLICE = N_NODES // N_CORES  # 6250 nodes per core for x@root
RCHUNK = 512


# ----------------------------------------------------------------- host prep

def _prep_inputs(x, edge_index, edge_attr, W1, b1, W2, b2, W3, b3, W4, b4,
                 root, bias):
    src_all = np.asarray(edge_index[0], np.int64)
    dst_all = np.asarray(edge_index[1], np.int64)
    attr_all = np.asarray(edge_attr, np.float32)
    x = np.asarray(x, np.float32)
    W1f = np.asarray(W1, np.float32)
    b1f = np.asarray(b1, np.float32)

    # weights: channel-major (o major, i minor) W4 with bias as extra row;
    # W3 gains a zero output column whose bias is 1.0 so h3[100] == 1 feeds
    # W4's bias row after relu.
    W4p = np.asarray(W4, np.float32).reshape(100, 16, 16).transpose(0, 2, 1).reshape(100, 256)
    b4p = np.asarray(b4, np.float32).reshape(16, 16).T.reshape(256)
    W4a = np.concatenate([W4p, b4p[None, :]], axis=0).astype(FP16)  # [101,256]
    W3a = np.concatenate([np.asarray(W3, np.float32),
                          np.zeros((100, 1), np.float32)], axis=1).astype(FP16)
    b3a = np.concatenate([np.asarray(b3, np.float32),
                          np.ones(1, np.float32)]).reshape(101, 1)
    roota = np.concatenate([np.asarray(root, np.float32),
                            np.asarray(bias, np.float32)[None, :]], axis=0).astype(FP16)
    const = {
        "W2": np.asarray(W2, np.float32).astype(FP16),
        "W3": W3a,
        "W4a": W4a,
        "b2": np.asarray(b2, np.float32).reshape(100, 1),
        "b3": b3a,
        "roota": roota,
    }

    in_maps = []
    node_maps = []
    for k in range(N_CORES):
        sl = slice(k * ESH, (k + 1) * ESH)
        dst = dst_all[sl]
        order = np.argsort(dst, kind="stable")
        dst_s = dst[order]
        src_s = src_all[sl][order]
        attr_s = attr_all[sl][order]

        starts = np.concatenate([[0], np.flatnonzero(np.diff(dst_s)) + 1])
        uniq = dst_s[starts]
        lens = np.diff(np.concatenate([starts, [ESH]]))
        node_maps.append((starts, uniq, lens))

        # layer 1 on the host: h1 = relu(attr@W1 + b1), shipped transposed
        h1 = np.maximum(attr_s @ W1f + b1f, 0.0)
        h1p = np.zeros((EP, 100), np.float32)
        h1p[:ESH] = h1
        h1T = np.ascontiguousarray(h1p.T).astype(FP16)  # [100, EP]

        xg = np.zeros((EP, 16), np.float32)
        xg[:ESH] = x[src_s]
        # edge e = ((b*4 + tt)*4 + g)*128 + p  ->  [NB, 128, (tt,g,c)]
        xgp = np.ascontiguousarray(
            xg.reshape(NB, TB, 4, P, 16).transpose(0, 3, 1, 2, 4)
        ).reshape(NB, P, TB * 4 * 16).astype(FP16)

        xsl = x[k * NSLICE:(k + 1) * NSLICE].T  # [16, NSLICE]
        xslT = np.ascontiguousarray(
            np.concatenate([xsl, np.ones((1, NSLICE), np.float32)], axis=0)
        ).astype(FP16)  # [17, NSLICE]

        in_maps.append(dict(const, h1T=h1T, xg=xgp, xslT=xslT))
    return in_maps, node_maps


# ------------------------------------------------------------ device program

_PROG_CACHE = {}


def build_program():
    if "nc" in _PROG_CACHE:
        return _PROG_CACHE["nc"]

    f32, fp16 = mybir.dt.float32, mybir.dt.float16

    nc = bacc.Bacc(None, target_bir_lowering=False, debug=True)
    h1T = nc.dram_tensor("h1T", [100, EP], fp16, kind="ExternalInput")
    xg = nc.dram_tensor("xg", [NB, P, TB * 64], fp16, kind="ExternalInput")
    xslT = nc.dram_tensor("xslT", [17, NSLICE], fp16, kind="ExternalInput")
    W2 = nc.dram_tensor("W2", [100, 100], fp16, kind="ExternalInput")
    W3 = nc.dram_tensor("W3", [100, 101], fp16, kind="ExternalInput")
    W4a = nc.dram_tensor("W4a", [101, 256], fp16, kind="ExternalInput")
    b2 = nc.dram_tensor("b2", [100, 1], f32, kind="ExternalInput")
    b3 = nc.dram_tensor("b3", [101, 1], f32, kind="ExternalInput")
    roota = nc.dram_tensor("roota", [17, 16], fp16, kind="ExternalInput")
    msgout = nc.dram_tensor("msgout", [NB, P, TB * 64], fp16, kind="ExternalOutput")
    rootp = nc.dram_tensor("rootp", [16, NSLICE], f32, kind="ExternalOutput")

    AT = mybir.ActivationFunctionType
    OP = mybir.AluOpType

    with tile.TileContext(nc) as tc, \
         nc.allow_low_precision(reason="fp16 intermediates, fp32 accumulation"):
        with tc.tile_pool(name="consts", bufs=1) as cp, \
             tc.tile_pool(name="io", bufs=4) as iop, \
             tc.tile_pool(name="work", bufs=4) as wp, \
             tc.tile_pool(name="ps2", bufs=2, space="PSUM") as pm2, \
             tc.tile_pool(name="ps3", bufs=2, space="PSUM") as pm3, \
             tc.tile_pool(name="psw", bufs=2, space="PSUM") as pw:

            W2sb = cp.tile([100, 100], fp16)
            W3sb = cp.tile([100, 101], fp16)
            W4sb = cp.tile([101, 256], fp16)
            b2sb = cp.tile([100, 1], f32)
            b3sb = cp.tile([101, 1], f32)
            rsb = cp.tile([17, 16], fp16)
            xssb = cp.tile([17, NSLICE], fp16)
            rout = cp.tile([16, NSLICE], f32)
            for t_sb, t_dr in ((W2sb, W2), (W3sb, W3), (W4sb, W4a),
                               (b2sb, b2), (b3sb, b3), (rsb, roota),
                               (xssb, xslT)):
                nc.sync.dma_start(t_sb[:], t_dr[:])

            for b in range(NB):
                h1_sb = iop.tile([100, TB * ETILE], fp16, tag="h1")
                nc.sync.dma_start(h1_sb[:], h1T[:, b * TB * ETILE:(b + 1) * TB * ETILE])
                xg_sb = iop.tile([P, TB, 4, 16], fp16, tag="xg")
                nc.sync.dma_start(
                    xg_sb[:].rearrange("p t g c -> p (t g c)"), xg[b])
                msg_sb = iop.tile([P, TB, 4, 16], fp16, tag="msg")

                for tt in range(TB):
                    h1sl = h1_sb[:, tt * ETILE:(tt + 1) * ETILE]
                    ps2 = pm2.tile([100, ETILE], f32, tag="mlp2")
                    nc.tensor.matmul(ps2[:], lhsT=W2sb[:], rhs=h1sl, start=True, stop=True)
                    h2 = wp.tile([100, ETILE], fp16, tag="h2")
                    nc.scalar.activation(h2[:], ps2[:], AT.Relu, bias=b2sb[:, 0:1])
                    ps3 = pm3.tile([101, ETILE], f32, tag="mlp3")
                    nc.tensor.matmul(ps3[:], lhsT=W3sb[:], rhs=h2[:], start=True, stop=True)
                    h3 = wp.tile([101, ETILE], fp16, tag="h3")
                    nc.scalar.activation(h3[:], ps3[:], AT.Relu, bias=b3sb[:, 0:1])

                    psw = pw.tile([P, 4, 256], f32, tag="w")
                    for g in range(4):
                        nc.tensor.matmul(
                            psw[:, g, :], lhsT=h3[:, g * P:(g + 1) * P],
                            rhs=W4sb[:], start=True, stop=True)

                    prod = wp.tile([P, 4, 16, 16], fp16, tag="prod")
                    nc.vector.tensor_tensor(
                        out=prod[:],
                        in0=psw[:].rearrange("p g (o i) -> p g o i", i=16),
                        in1=xg_sb[:, tt, :, :][:, :, None, :].to_broadcast([P, 4, 16, 16]),
                        op=OP.mult)
                    # 16 -> 1 pairwise fp16 add tree (level 1 on Pool, rest DVE)
                    t1 = wp.tile([P, 4, 16, 8], fp16, tag="t1")
                    nc.gpsimd.tensor_tensor(
                        out=t1[:], in0=prod[:, :, :, 0:8], in1=prod[:, :, :, 8:16],
                        op=OP.add)
                    t2 = wp.tile([P, 4, 16, 4], fp16, tag="t2")
                    nc.vector.tensor_tensor(
                        out=t2[:], in0=t1[:, :, :, 0:4], in1=t1[:, :, :, 4:8],
                        op=OP.add)
                    t3 = wp.tile([P, 4, 16, 2], fp16, tag="t3")
                    nc.gpsimd.tensor_tensor(
                        out=t3[:], in0=t2[:, :, :, 0:2], in1=t2[:, :, :, 2:4],
                        op=OP.add)
                    nc.vector.tensor_tensor(
                        out=msg_sb[:, tt, :, :], in0=t3[:, :, :, 0], in1=t3[:, :, :, 1],
                        op=OP.add)

                nc.sync.dma_start(
                    msgout[b], msg_sb[:].rearrange("p t g c -> p (t g c)"))

            # x@root + bias, column-streamed: rootp[o, n] = roota^T @ xslT
            for c in range(math.ceil(NSLICE / RCHUNK)):
                n0 = c * RCHUNK
                w = min(RCHUNK, NSLICE - n0)
                psr = pm2.tile([16, RCHUNK], f32, tag="mlp2")
                nc.tensor.matmul(psr[:, 0:w], lhsT=rsb[:], rhs=xssb[:, n0:n0 + w],
                                 start=True, stop=True)
                nc.scalar.copy(rout[:, n0:n0 + w], psr[:, 0:w])
            nc.sync.dma_start(rootp[:], rout[:])

    nc.compile()
    _PROG_CACHE["nc"] = nc
    return nc


# ------------------------------------------------------------------- driver

def _combine(results, node_maps):
    acc = np.zeros((N_NODES, 16), np.float32)
    cnt = np.zeros(N_NODES, np.float32)
    rootparts = []
    for r, (starts, uniq, lens) in zip(results, node_maps):
        m = np.asarray(r["msgout"]).reshape(NB, P, TB, 4, 16) \
            .transpose(0, 2, 3, 1, 4).reshape(EP, 16)[:ESH].astype(np.float32)
        acc[uniq] += np.add.reduceat(m, starts, axis=0)
        cnt[uniq] += lens
        rootparts.append(np.asarray(r["rootp"]).T)
    agg = acc / np.maximum(cnt, 1.0)[:, None]
    return np.concatenate(rootparts, axis=0) + agg


def _run(inputs, trace=False):
    in_maps, node_maps = _prep_inputs(**inputs)
    nc = build_program()
    res = run_bass_kernel_spmd(nc, in_maps, list(range(N_CORES)), trace=trace)
    out = _combine(res.results, node_maps)
    return out.astype(np.float32), res


def kernel(**inputs) -> np.ndarray:
    out, _ = _run(inputs, trace=False)
    return out


# revision 18
# speedup vs baseline: 5.1799x; 5.1799x over previous
"""GNO message-passing kernel for Trainium2 (8 NeuronCores, edge-parallel).

Math (matches the reference):
    h  = relu(relu(relu(ea@W1+b1)@W2+b2)@W3+b3)
    w  = (h@W4+b4).reshape(E,16,16)
    msg= einsum('ei,eio->eo', x[src], w)
    agg= segment_mean(msg, dst, N)
    out= x@root + agg + bias

Strategy (v3):
  - Edges are split into 8 contiguous shards (one per core) and sorted by
    dst on the host.  The host pre-gathers x[src], and also computes the
    first MLP layer h1 = relu(ea@W1+b1) (cheap BLAS), so the device sees
    dense fp16 streams: h1 (tile-major) and gathered source features.
  - Per 512-edge tile on-device: W2/W3/W4 on TensorE (bias-as-extra-row
    trick for layer 4), relu2/relu3 on Scalar (Act), per-edge einsum:
    DVE broadcast-multiply from PSUM, then a pairwise fp16 add-tree for
    the 16->1 reduction -- first level on GpSimd (Pool), rest on DVE
    where the all-SBUF fp16 2x fast path applies.  Per-edge messages are
    written back densely in fp16 -- no on-device scatter at all.
  - x@root+bias is column-streamed through one stationary [17,16] load.
  - Host: np.add.reduceat over dst-sorted runs, divide by counts, add
    the root part.
"""

import math
import numpy as np

import concourse.bass as bass
import concourse.bacc as bacc
import concourse.mybir as mybir
import concourse.tile as tile
from concourse.bass_utils import run_bass_kernel_spmd

FP16 = np.float16

N_NODES = 50000
N_EDGES = 800000
N_CORES = 8
ETILE = 512
TB = 4                      # tiles per DMA batch
P = 128
ESH = N_EDGES // N_CORES    # 100000 edges per core
T = math.ceil(math.ceil(ESH / ETILE) / TB) * TB  # tiles, multiple of TB
NB = T // TB                # 49 batches
EP = T * ETILE              # 100352 padded edges
NSLICE = N_NODES // N_CORES  # 6250 nodes per core for x@root
RCHUNK = 512


# ----------------------------------------------------------------- host prep

def _prep_inputs(x, edge_index, edge_attr, W1, b1, W2, b2, W3, b3, W4, b4,
                 root, bias):
    src_all = np.asarray(edge_index[0], np.int64)
    dst_all = np.asarray(edge_index[1], np.int64)
    attr_all = np.asarray(edge_attr, np.float32)
    x = np.asarray(x, np.float32)
    W1f = np.asarray(W1, np.float32)
    b1f = np.asarray(b1, np.float32)

    # weights: channel-major (o major, i minor) W4 with bias as extra row;
    # W3 gains a zero output column whose bias is 1.0 so h3[100] == 1 feeds
    # W4's bias row after relu.
    W4p = np.asarray(W4, np.float32).reshape(100, 16, 16).transpose(0, 2, 1).reshape(100, 256)
    b4p = np.asarray(b4, np.float32).reshape(16, 16).T.reshape(256)
    W4a = np.concatenate([W4p, b4p[None, :]], axis=0).astype(FP16)  # [101,256]
    W3a = np.concatenate([np.asarray(W3, np.float32),
                          np.zeros((100, 1), np.float32)], axis=1).astype(FP16)
    b3a = np.concatenate([np.asarray(b3, np.float32),
                          np.ones(1, np.float32)]).reshape(101, 1)
    roota = np.concatenate([np.asarray(root, np.float32),
                            np.asarray(bias, np.float32)[None, :]], axis=0).astype(FP16)
    const = {
        "W2": np.asarray(W2, np.float32).astype(FP16),
        "W3": W3a,
        "W4a": W4a,
        "b2": np.asarray(b2, np.float32).reshape(100, 1),
        "b3": b3a,
        "roota": roota,
    }

    in_maps = []
    node_maps = []
    for k in range(N_CORES):
        sl = slice(k * ESH, (k + 1) * ESH)
        dst = dst_all[sl]
        order = np.argsort(dst, kind="stable")
        dst_s = dst[order]
        src_s = src_all[sl][order]
        attr_s = attr_all[sl][order]

        starts = np.concatenate([[0], np.flatnonzero(np.diff(dst_s)) + 1])
        uniq = dst_s[starts]
        lens = np.diff(np.concatenate([starts, [ESH]]))
        node_maps.append((starts, uniq, lens))

        # layer 1 on the host: h1 = relu(attr@W1 + b1), shipped transposed
        h1 = np.maximum(attr_s @ W1f + b1f, 0.0)
        h1p = np.zeros((EP, 100), np.float32)
        h1p[:ESH] = h1
        h1T = np.ascontiguousarray(h1p.T).astype(FP16)  # [100, EP]

        xg = np.zeros((EP, 16), np.float32)
        xg[:ESH] = x[src_s]
        # edge e = ((b*4 + tt)*4 + g)*128 + p  ->  [NB, 128, (tt,g,c)]
        xgp = np.ascontiguousarray(
            xg.reshape(NB, TB, 4, P, 16).transpose(0, 3, 1, 2, 4)
        ).reshape(NB, P, TB * 4 * 16).astype(FP16)

        xsl = x[k * NSLICE:(k + 1) * NSLICE].T  # [16, NSLICE]
        xslT = np.ascontiguousarray(
            np.concatenate([xsl, np.ones((1, NSLICE), np.float32)], axis=0)
        ).astype(FP16)  # [17, NSLICE]

        in_maps.append(dict(const, h1T=h1T, xg=xgp, xslT=xslT))
    return in_maps, node_maps


# ------------------------------------------------------------ device program

_PROG_CACHE = {}


TREE = "conv2"  # einsum variant: Act converts 2 W4 groups, DVE 2x multiply + add tree


def build_program(rep=1, tree=None):
    tree = TREE if tree is None else tree
    key = (rep, tree)
    if key in _PROG_CACHE:
        return _PROG_CACHE[key]

    f32, fp16 = mybir.dt.float32, mybir.dt.float16

    nc = bacc.Bacc(None, target_bir_lowering=False, debug=True)
    h1T = nc.dram_tensor("h1T", [100, EP], fp16, kind="ExternalInput")
    xg = nc.dram_tensor("xg", [NB, P, TB * 64], fp16, kind="ExternalInput")
    xslT = nc.dram_tensor("xslT", [17, NSLICE], fp16, kind="ExternalInput")
    W2 = nc.dram_tensor("W2", [100, 100], fp16, kind="ExternalInput")
    W3 = nc.dram_tensor("W3", [100, 101], fp16, kind="ExternalInput")
    W4a = nc.dram_tensor("W4a", [101, 256], fp16, kind="ExternalInput")
    b2 = nc.dram_tensor("b2", [100, 1], f32, kind="ExternalInput")
    b3 = nc.dram_tensor("b3", [101, 1], f32, kind="ExternalInput")
    roota = nc.dram_tensor("roota", [17, 16], fp16, kind="ExternalInput")
    msgout = nc.dram_tensor("msgout", [NB, P, TB * 64], fp16, kind="ExternalOutput")
    rootp = nc.dram_tensor("rootp", [16, NSLICE], f32, kind="ExternalOutput")

    AT = mybir.ActivationFunctionType
    OP = mybir.AluOpType

    with tile.TileContext(nc) as tc, \
         nc.allow_low_precision(reason="fp16 intermediates, fp32 accumulation"):
        with tc.tile_pool(name="consts", bufs=1) as cp, \
             tc.tile_pool(name="io", bufs=3) as iop, \
             tc.tile_pool(name="work", bufs=4) as wp, \
             tc.tile_pool(name="ps2", bufs=2, space="PSUM") as pm2, \
             tc.tile_pool(name="ps3", bufs=2, space="PSUM") as pm3, \
             tc.tile_pool(name="psw", bufs=2, space="PSUM") as pw:

            W2sb = cp.tile([100, 100], fp16)
            W3sb = cp.tile([100, 101], fp16)
            W4sb = cp.tile([101, 256], fp16)
            b2sb = cp.tile([100, 1], f32)
            b3sb = cp.tile([101, 1], f32)
            rsb = cp.tile([17, 16], fp16)
            xssb = cp.tile([17, NSLICE], fp16)
            rout = cp.tile([16, NSLICE], f32)
            for t_sb, t_dr in ((W2sb, W2), (W3sb, W3), (W4sb, W4a),
                               (b2sb, b2), (b3sb, b3), (rsb, roota),
                               (xssb, xslT)):
                nc.sync.dma_start(t_sb[:], t_dr[:])

            for _r in range(rep):
              for b in range(NB):
                h1_sb = iop.tile([100, TB * ETILE], fp16, tag="h1")
                nc.sync.dma_start(h1_sb[:], h1T[:, b * TB * ETILE:(b + 1) * TB * ETILE])
                xg_sb = iop.tile([P, TB, 4, 16], fp16, tag="xg")
                nc.sync.dma_start(
                    xg_sb[:].rearrange("p t g c -> p (t g c)"), xg[b])
                msg_sb = iop.tile([P, TB, 4, 16], fp16, tag="msg")

                for tt in range(TB):
                    h1sl = h1_sb[:, tt * ETILE:(tt + 1) * ETILE]
                    ps2 = pm2.tile([100, ETILE], f32, tag="mlp2")
                    nc.tensor.matmul(ps2[:], lhsT=W2sb[:], rhs=h1sl, start=True, stop=True)
                    h2 = wp.tile([100, ETILE], fp16, tag="h2")
                    if tree.endswith("r2d"):
                        # relu2 on DVE: (ps2 + b2) then max(0, .)
                        nc.vector.tensor_scalar(
                            out=h2[:], in0=ps2[:], scalar1=b2sb[:, 0:1],
                            scalar2=0.0, op0=OP.add, op1=OP.max)
                    else:
                        nc.scalar.activation(h2[:], ps2[:], AT.Relu, bias=b2sb[:, 0:1])
                    ps3 = pm3.tile([101, ETILE], f32, tag="mlp3")
                    nc.tensor.matmul(ps3[:], lhsT=W3sb[:], rhs=h2[:], start=True, stop=True)
                    h3 = wp.tile([101, ETILE], fp16, tag="h3")
                    nc.scalar.activation(h3[:], ps3[:], AT.Relu, bias=b3sb[:, 0:1])

                    psw = pw.tile([P, 4, 256], f32, tag="w")
                    for g in range(4):
                        nc.tensor.matmul(
                            psw[:, g, :], lhsT=h3[:, g * P:(g + 1) * P],
                            rhs=W4sb[:], start=True, stop=True)

                    prod = wp.tile([P, 4, 16, 16], fp16, tag="prod")
                    # convN: Act converts the last N W4-output groups to
                    # fp16 SBUF so DVE's multiply of those groups runs in
                    # the 2x all-16-bit mode
                    ng = int(tree[4]) if tree.startswith("conv") else 0
                    if ng:
                        w16 = wp.tile([P, ng, 256], fp16, tag="w16")
                        nc.scalar.copy(w16[:], psw[:, 4 - ng:4, :])
                        if ng < 4:
                            nc.vector.tensor_tensor(
                                out=prod[:, 0:4 - ng],
                                in0=psw[:, 0:4 - ng, :].rearrange(
                                    "p g (o i) -> p g o i", i=16),
                                in1=xg_sb[:, tt, 0:4 - ng, :][:, :, None, :]
                                    .to_broadcast([P, 4 - ng, 16, 16]),
                                op=OP.mult)
                        nc.vector.tensor_tensor(
                            out=prod[:, 4 - ng:4],
                            in0=w16[:].rearrange("p g (o i) -> p g o i", i=16),
                            in1=xg_sb[:, tt, 4 - ng:4, :][:, :, None, :]
                                .to_broadcast([P, ng, 16, 16]),
                            op=OP.mult)
                    else:
                        nc.vector.tensor_tensor(
                            out=prod[:],
                            in0=psw[:].rearrange("p g (o i) -> p g o i", i=16),
                            in1=xg_sb[:, tt, :, :][:, :, None, :].to_broadcast([P, 4, 16, 16]),
                            op=OP.mult)
                    if tree == "r":
                        # single-instruction grouped reduce on DVE
                        nc.vector.reduce_sum(
                            out=msg_sb[:, tt, :, :], in_=prod[:],
                            axis=mybir.AxisListType.X)
                    else:
                        # 16 -> 1 pairwise fp16 add tree across Pool/DVE
                        lvls = "dddd" if tree.startswith("conv") else tree
                        eng = {"p": nc.gpsimd, "d": nc.vector}
                        cur = prod
                        for lvl, width in enumerate((8, 4, 2, 1)):
                            e = eng[lvls[lvl]]
                            if width == 1:
                                e.tensor_tensor(
                                    out=msg_sb[:, tt, :, :], in0=cur[:, :, :, 0],
                                    in1=cur[:, :, :, 1], op=OP.add)
                            else:
                                nxt = wp.tile([P, 4, 16, width], fp16,
                                              tag=f"t{lvl + 1}")
                                e.tensor_tensor(
                                    out=nxt[:], in0=cur[:, :, :, 0:width],
                                    in1=cur[:, :, :, width:2 * width], op=OP.add)
                                cur = nxt

                nc.sync.dma_start(
                    msgout[b], msg_sb[:].rearrange("p t g c -> p (t g c)"))

            # x@root + bias, column-streamed: rootp[o, n] = roota^T @ xslT
            for c in range(rep * math.ceil(NSLICE / RCHUNK)):
                c = c % math.ceil(NSLICE / RCHUNK)
                n0 = c * RCHUNK
                w = min(RCHUNK, NSLICE - n0)
                psr = pm2.tile([16, RCHUNK], f32, tag="mlp2")
                nc.tensor.matmul(psr[:, 0:w], lhsT=rsb[:], rhs=xssb[:, n0:n0 + w],
                                 start=True, stop=True)
                nc.scalar.copy(rout[:, n0:n0 + w], psr[:, 0:w])
            nc.sync.dma_start(rootp[:], rout[:])

    nc.compile()
    _PROG_CACHE[key] = nc
    return nc


# ------------------------------------------------------------------- driver

def _combine(results, node_maps):
    acc = np.zeros((N_NODES, 16), np.float32)
    cnt = np.zeros(N_NODES, np.float32)
    rootparts = []
    for r, (starts, uniq, lens) in zip(results, node_maps):
        m = np.asarray(r["msgout"]).reshape(NB, P, TB, 4, 16) \
            .transpose(0, 2, 3, 1, 4).reshape(EP, 16)[:ESH].astype(np.float32)
        acc[uniq] += np.add.reduceat(m, starts, axis=0)
        cnt[uniq] += lens
        rootparts.append(np.asarray(r["rootp"]).T)
    agg = acc / np.maximum(cnt, 1.0)[:, None]
    return np.concatenate(rootparts, axis=0) + agg


def _run(inputs, trace=False):
    in_maps, node_maps = _prep_inputs(**inputs)
    nc = build_program()
    res = run_bass_kernel_spmd(nc, in_maps, list(range(N_CORES)), trace=trace)
    out = _combine(res.results, node_maps)
    return out.astype(np.float32), res


def kernel(**inputs) -> np.ndarray:
    out, _ = _run(inputs, trace=False)
    return out
